# revision 13
# baseline (speedup 1.0000x reference)
"""Two-layer GCN forward (GCNConv -> relu -> GCNConv -> log_softmax) on 8
Trainium2 NeuronCores - single monolithic SPMD program.

Design (pull aggregation via bulk dma_gather):
  - Nodes are in-degree-sorted globally and dealt round-robin to 8 cores;
    core-local layout is (p, g), lr = g*128 + p, global table id
    tid = m*NPAD + p*G + g.
  - Per layer the cores publish t = dis*h as a bf16 [NPAD,16] slice,
    AllGather it into the full [NTBL,16] table, and expand it locally into a
    256B-strided [NTBL,128] bf16 table (payload cols 0:16; rest junk, never
    read) because dma_gather requires 256B elements/strides and int16 ids.
  - Each core then pulls its targets' in-neighbor rows with big dma_gather
    instructions (thousands of rows each, Q7 "mlp" ucode): slots are
    organized per (source-quarter q, target group g) with uniform K(g,q)
    columns (shared schedule, group max over the 1024-node degree band);
    pad slots point at a dummy (zero) row. Gathered chunks are reduced with
    run-length uniform-K DVE tensor_reduce into per-quarter partials, then
    summed. Gathers round-robin over 4 SWDGE queues (queue q runs on Q7
    core pair 2q/2q+1; HW overlaps 2 in flight) with a 6-deep gather-buffer
    rotation so the random-HBM descriptor drains (~95ns/desc/engine, the
    real bottleneck) pipeline across chunks.
  - norm factorizes: table rows are dis_u*h_u; epilogue does
    dis_v*(agg + t_v) (+b). Self loops handled densely; dis = rsqrt(deg+1)
    comes from the host, as do all int16 slot tables.

kernel(**inputs) takes full unsharded inputs, returns the full [N, 40] out.
"""

import sys

import numpy as np

try:
    import concourse.bass as bass
except ImportError:  # pragma: no cover
    sys.path.insert(0, "/opt/trn_rl_repo")
    import concourse.bass as bass

import concourse.bacc as bacc
import concourse.tile as tile
from concourse import library_config, mybir
from concourse.bass_utils import run_bass_kernel_spmd
from concourse.masks import make_identity

F32 = mybir.dt.float32
BF16 = mybir.dt.bfloat16
I16 = mybir.dt.int16
P = 128
NCORES = 8
NQ = 4  # source-table quarters (int16 row-id windows)
CCH = 60  # max gather-chunk columns (tokens = 128*CCH); >=16k tokens hangs the ucode


def _plan(edge_index, N):
    row = np.asarray(edge_index[0], dtype=np.int64)
    col = np.asarray(edge_index[1], dtype=np.int64)

    NP_ = -(-N // NCORES)
    G = -(-NP_ // P)
    NPAD = G * P
    NTBL = NCORES * NPAD

    # overlapping int16 windows over the table: width WW, stride chosen so
    # most rows fall in two windows (per-edge window choice -> water-fill
    # balancing of per-target slot counts).
    WW = min(NTBL, 32752)
    if NTBL <= WW:
        W = 1
        stride = NTBL
        bases = np.array([0], dtype=np.int64)
    else:
        stride = WW // 2
        nb = -(-NTBL // stride)  # bands; band j rows -> window pair (j, j+1)
        W = nb + 1
        bases = np.clip((np.arange(W) - 1) * stride, 0, NTBL - WW)
    assert (bases + WW <= NTBL).all() and bases[-1] + WW >= NTBL

    deg_in = np.bincount(col, minlength=N).astype(np.float64)
    dis = 1.0 / np.sqrt(deg_in + 1.0)

    order = np.argsort(-deg_in, kind="stable")
    rank = np.empty(N, dtype=np.int64)
    rank[order] = np.arange(N)

    m_of = rank % NCORES
    lr_of = rank // NCORES
    g_of = lr_of // P
    p_of = lr_of % P
    tid_of = m_of * NPAD + p_of * G + g_of

    # dummy (zero) rows: one per window
    used = np.zeros(NTBL, dtype=bool)
    used[tid_of] = True
    free_rows = np.nonzero(~used)[0]
    assert free_rows.size > 0, "no dummy rows"
    dummy_of_w = np.empty(W, dtype=np.int64)
    for w in range(W):
        inw = free_rows[(free_rows >= bases[w]) & (free_rows < bases[w] + WW)]
        assert inw.size > 0, f"no dummy row in window {w}"
        dummy_of_w[w] = inw[0]

    # per-edge legal windows: band j = r//stride -> pair (j, j+1); with
    # stride = WW//2 and duplicated end windows every row has both legal.
    r = tid_of[row]
    if W == 1:
        w0 = np.zeros(len(r), dtype=np.int64)
        w1 = w0.copy()
    else:
        w0 = r // stride
        w1 = w0 + 1
        assert (r >= bases[w0]).all() and (r < bases[w0] + WW).all()
        assert (r >= bases[w1]).all() and (r < bases[w1] + WW).all()
    flex = w0 != w1  # choice between w0 and w0+1

    # water-fill per target: forced counts f[v,w], flex counts y[v,w0]
    deg = np.bincount(col, minlength=N)
    f = np.zeros((W, N), dtype=np.int64)
    y = np.zeros((W, N), dtype=np.int64)
    for w in range(W):
        f[w] = np.bincount(col[(~flex) & (w1 == w)], minlength=N)
        y[w] = np.bincount(col[flex & (w0 == w)], minlength=N)
    T = -(-deg // W)
    keep = np.zeros((W, N), dtype=np.int64)  # flex edges kept at lower window
    carry = np.zeros(N, dtype=np.int64)
    for w in range(W):
        load = f[w] + carry
        lvl = np.maximum(T, (load + y[w] + 1) // 2)  # split band spikes
        t_w = np.clip(lvl - load, 0, y[w])
        keep[w] = t_w
        carry = y[w] - t_w
    # per-edge final window: rank within (target, w0) among flex edges
    e_id = np.arange(len(row))
    fkey = np.lexsort((e_id, w0, col))
    fsel = flex[fkey]
    fk = fkey[fsel]
    grp = col[fk] * W + w0[fk]
    ch = np.empty(len(fk), dtype=bool)
    if len(fk):
        ch[0] = True
        ch[1:] = grp[1:] != grp[:-1]
        st = np.nonzero(ch)[0]
        gi = np.cumsum(ch) - 1
        rk = np.arange(len(fk)) - st[gi]
        kept = rk < keep[w0[fk], col[fk]]
        wfin = np.where(~flex, w1, 0)
        wfin[fk] = np.where(kept, w0[fk], w0[fk] + 1)
    else:
        wfin = w1.copy()

    # per (target, window) final counts -> shared K schedule
    def _counts(wf):
        cw = np.zeros((W, N), dtype=np.int64)
        for w in range(W):
            cw[w] = np.bincount(col[wf == w], minlength=N)
        K = np.zeros((W, G), dtype=np.int64)
        for w in range(W):
            cpad = np.zeros(NCORES * NPAD, dtype=np.int64)
            cpad[:N] = cw[w][order]
            K[w] = cpad.reshape(G, NCORES * P).max(axis=1)
        return cw, K

    cw, K = _counts(wfin)
    if W > 1:
        # band-max refinement: move one edge per (target, window) away from
        # ceiling windows into the adjacent legal window with headroom
        band_of = np.empty(N, dtype=np.int64)
        band_of[order] = np.arange(N) // (NCORES * P)
        alt = np.where(wfin == w0, w0 + 1, w0)
        for _ in range(4):
            Kv = K[:, band_of]
            mov = (cw[wfin, col] >= Kv[wfin, col]) & (
                Kv[alt, col] - cw[alt, col] >= 2
            )
            sel = np.nonzero(mov)[0]
            if sel.size == 0:
                break
            key = col[sel] * W + wfin[sel]
            o = np.argsort(key, kind="stable")
            sel, key = sel[o], key[o]
            first = np.ones(len(sel), dtype=bool)
            first[1:] = key[1:] != key[:-1]
            wfin[sel[first]] = alt[sel[first]]
            cw, K = _counts(wfin)

    # column layout / chunks (uniform-K runs, chunks of <= CCH columns)
    chunks = []  # (w, ncols, [(g0, g1, K, colbase)...])
    colbase_of = np.zeros((W, G), dtype=np.int64)
    for w in range(W):
        g = 0
        while g < G:
            ccols = 0
            cruns = []
            while g < G and ccols + K[w, g] <= CCH:
                k = int(K[w, g])
                g1 = g
                while g1 < G and K[w, g1] == k and ccols + (g1 - g + 1) * k <= CCH:
                    colbase_of[w, g1] = ccols + (g1 - g) * k
                    g1 += 1
                if k > 0:
                    cruns.append((g, g1, k, ccols))
                    ccols += (g1 - g) * k
                g = g1
            if not cruns and g < G:  # single group wider than CCH
                k = int(K[w, g])
                colbase_of[w, g] = 0
                cruns.append((g, g + 1, k, 0))
                ccols = k
                g += 1
            if cruns:
                chunks.append((w, ccols, cruns))
    TOTTOK = sum(c[1] for c in chunks) * P

    chunk_off = []
    off = 0
    for w, ccols, cruns in chunks:
        chunk_off.append(off)
        off += 8 * ccols
    TOTCOL = off

    idxt = np.zeros((NCORES, 16, TOTCOL), dtype=np.int16)
    for ci, (w, ccols, cruns) in enumerate(chunks):
        base = chunk_off[ci]
        j = np.arange(128 * ccols)
        idxt[:, (j % 16), base + j // 16] = dummy_of_w[w] - bases[w]
    # place real edges
    mv, pv, gv = m_of[col], p_of[col], g_of[col]
    srcl = tid_of[row] - bases[wfin]
    assert (srcl >= 0).all() and (srcl < WW).all()
    ekey = np.lexsort((row, gv, pv, wfin, mv))
    smv, spv, sgv, swv = mv[ekey], pv[ekey], gv[ekey], wfin[ekey]
    ssrc = srcl[ekey]
    grp = ((smv * P + spv) * G + sgv) * W + swv
    changes = np.empty(len(grp), dtype=bool)
    changes[0] = True
    changes[1:] = grp[1:] != grp[:-1]
    starts = np.nonzero(changes)[0]
    gid = np.cumsum(changes) - 1
    k_of = np.arange(len(grp)) - starts[gid]
    chunk_of_wg = np.zeros((W, G), dtype=np.int64)
    for ci, (w, ccols, cruns) in enumerate(chunks):
        for (g0, g1, k, cb) in cruns:
            chunk_of_wg[w, g0:g1] = ci
    cidx = chunk_of_wg[swv, sgv]
    colc = colbase_of[swv, sgv] + k_of
    j = colc * 128 + spv
    coff = np.asarray(chunk_off)[cidx]
    idxt[smv, j % 16, coff + j // 16] = ssrc.astype(np.int16)

    return dict(
        NP=NP_, G=G, NPAD=NPAD, NTBL=NTBL, W=W, WW=WW, bases=bases,
        TOTCOL=TOTCOL, TOTTOK=TOTTOK, chunks=chunks, chunk_off=chunk_off,
        order=order, dis=dis, m_of=m_of, p_of=p_of, g_of=g_of, idxt=idxt,
    )


def _build(cfg, G, plan):
    IN_CH, HID, OUT_CH = cfg["IN_CH"], cfg["HID"], cfg["OUT_CH"]
    NPAD = G * P
    NTBL, W, WW = plan["NTBL"], plan["W"], plan["WW"]
    bases = plan["bases"]
    TOTCOL = plan["TOTCOL"]
    chunks, chunk_off = plan["chunks"], plan["chunk_off"]
    MAXC = max(c[1] for c in chunks)
    NC_IN = IN_CH // P
    XCHUNK = next(d for d in range(1, G + 1) if G % d == 0 and G // d <= 16)
    GH = G // XCHUNK

    nc = bacc.Bacc(None, num_devices=NCORES, num_swdge_queues=4)

    xt_d = nc.dram_tensor("xt", [IN_CH, NPAD], BF16, kind="ExternalInput")
    w1_d = nc.dram_tensor("w1", [P, NC_IN * HID], BF16, kind="ExternalInput")
    dis_d = nc.dram_tensor("dis", [P, G], F32, kind="ExternalInput")
    dmask_d = nc.dram_tensor("dmask", [P, G], F32, kind="ExternalInput")
    b1_d = nc.dram_tensor("b1", [1, HID], F32, kind="ExternalInput")
    w2_d = nc.dram_tensor("w2", [HID, OUT_CH], F32, kind="ExternalInput")
    b2_d = nc.dram_tensor("b2", [1, OUT_CH], F32, kind="ExternalInput")
    idx_d = nc.dram_tensor("idxt", [P, TOTCOL], I16, kind="ExternalInput")
    out_d = nc.dram_tensor("out", [P, G * OUT_CH], F32, kind="ExternalOutput")

    groups = [list(range(NCORES))]

    with tile.TileContext(nc) as tc:
        nc.gpsimd.load_library(library_config.mlp)
        with (
            tc.tile_pool(name="const", bufs=1) as const,
            tc.tile_pool(name="persist", bufs=1) as persist,
            tc.tile_pool(name="small", bufs=2) as small,
            tc.tile_pool(name="dram", bufs=1, space="DRAM") as dram,
            tc.tile_pool(name="psH", bufs=4, space="PSUM") as psH,
            tc.tile_pool(name="psT", bufs=2, space="PSUM") as psT,
            tc.tile_pool(name="psO", bufs=2, space="PSUM") as psO,
        ):
            # ---- constants ----
            ident = const.tile([P, P], F32)
            make_identity(nc, ident[:])
            warm = psT.tile([P, P], F32, tag="pt")
            nc.tensor.transpose(warm[:], ident[:], ident[:])
            w1sb = const.tile([P, NC_IN, HID], BF16)
            nc.sync.dma_start(
                w1sb[:], w1_d[:].rearrange("k (c f) -> k c f", f=HID)
            )
            w2st = small.tile([HID, OUT_CH], F32, tag="vts")
            nc.sync.dma_start(w2st[:], w2_d[:])
            w2sb = const.tile([HID, OUT_CH], F32)
            nc.vector.tensor_copy(w2sb[:], w2st[:])
            b1sb = const.tile([P, HID], F32)
            nc.sync.dma_start(b1sb[:], b1_d[:].to_broadcast([P, HID]))
            b2sb = const.tile([P, OUT_CH], F32)
            nc.sync.dma_start(b2sb[:], b2_d[:].to_broadcast([P, OUT_CH]))
            dis = const.tile([P, G], F32)
            nc.sync.dma_start(dis[:], dis_d[:])
            dmask = const.tile([P, G], F32)
            nc.sync.dma_start(dmask[:], dmask_d[:])

            def dis_bc(F):
                return (
                    dis[:]
                    .rearrange("p (g u) -> p g u", u=1)
                    .to_broadcast([P, G, F])
                )

            NBLK = min(8, G)
            GBLK = [
                (b * G // NBLK, (b + 1) * G // NBLK)
                for b in range(NBLK)
                if (b + 1) * G // NBLK > b * G // NBLK
            ]

            def dis_blk(t, g0, g1, F):
                return (
                    t[:, g0:g1]
                    .rearrange("p (g u) -> p g u", u=1)
                    .to_broadcast([P, g1 - g0, F])
                )

            def sl(t, g0, g1, F):
                return t[:, g0 * F : g1 * F].rearrange(
                    "p (g f) -> p g f", f=F
                )

            # ---- phase A: H = X @ W1 via pretransposed bf16 panels ----
            H = persist.tile([P, G * HID], F32, tag="H")
            with tc.tile_pool(name="xp", bufs=2) as xpp:
                for ch in range(XCHUNK):
                    xp = xpp.tile([P, NC_IN, GH * P], BF16, tag="xp")
                    nc.sync.dma_start(
                        xp[:],
                        xt_d[:, ch * GH * P : (ch + 1) * GH * P].rearrange(
                            "(c k) n -> k c n", k=P
                        ),
                    )
                    for gl in range(GH):
                        g = ch * GH + gl
                        hps = psH.tile([P, HID], F32, tag="hps")
                        for c in range(NC_IN):
                            nc.tensor.matmul(
                                hps[:],
                                lhsT=xp[:, c, gl * P : (gl + 1) * P],
                                rhs=w1sb[:, c, :],
                                start=(c == 0),
                                stop=(c == NC_IN - 1),
                            )
                        nc.vector.tensor_copy(
                            H[:, g * HID : (g + 1) * HID], hps[:]
                        )

            def publish(tsrc, layer):
                """t [P,G*HID] f32 -> bf16 slice -> AllGather -> 256B table."""
                tbf = small.tile([P, G * HID], BF16, tag="tbf")
                nc.vector.tensor_copy(tbf[:], tsrc[:])
                bounce = dram.tile([NPAD, HID], BF16, tag=f"bounce{layer}")
                nc.sync.dma_start(
                    bounce[:].rearrange("(p g) f -> p (g f)", p=P), tbf[:]
                )
                tbl = dram.tile(
                    [NTBL, HID], BF16, tag=f"tbl{layer}", addr_space="Shared"
                )
                nc.gpsimd.collective_compute(
                    "AllGather",
                    mybir.AluOpType.bypass,
                    ins=[bounce[:].opt()],
                    outs=[tbl[:].opt()],
                    replica_groups=groups,
                )
                tbl4 = dram.tile([NTBL, P], BF16, tag=f"tbl4{layer}")
                # window-aligned pieces: piece 0 covers window 0 exactly so
                # its gathers can start before the rest is expanded
                cuts = [0, min(WW, NTBL)]
                while cuts[-1] < NTBL:
                    cuts.append(min(cuts[-1] + 33800, NTBL))
                for lo, hi in zip(cuts[:-1], cuts[1:]):
                    nc.sync.dma_start(
                        tbl4[lo:hi, :HID], tbl[lo:hi, :]
                    )
                return tbl4

            def gather_reduce(tbl4, agg_tag):
                agg4 = persist.tile([P, G * W * HID], F32, tag="agg4")
                nc.vector.memset(agg4[:], 0.0)
                a4 = agg4[:].rearrange("p (g q f) -> p g q f", q=W, f=HID)
                with (
                    tc.tile_pool(name="gd", bufs=1) as gdp,
                    tc.tile_pool(name="ix", bufs=2) as ixp,
                ):
                    for ci, (q, ccols, cruns) in enumerate(chunks):
                        T = P * ccols
                        ist = ixp.tile([P, 8 * MAXC], I16, tag=f"ist{ci % 4}")
                        nc.sync.dma_start(
                            ist[:, : 8 * ccols],
                            idx_d[:, chunk_off[ci] : chunk_off[ci] + 8 * ccols],
                        )
                        gd = gdp.tile([P, MAXC, P], BF16, tag=f"gd{ci % 6}")
                        nc.gpsimd.dma_gather(
                            gd[:, :ccols, :],
                            tbl4[int(bases[q]) : int(bases[q]) + WW, :],
                            ist[:, : 8 * ccols],
                            T,
                            T,
                            P,
                            single_packet=False,
                            queue_num=ci % 4,
                        )
                        for (g0, g1, k, cb) in cruns:
                            nc.vector.tensor_reduce(
                                out=a4[:, g0:g1, q, :],
                                in_=gd[:, cb : cb + (g1 - g0) * k, :HID].rearrange(
                                    "p (g k) f -> p g f k", k=k
                                ),
                                axis=mybir.AxisListType.X,
                                op=mybir.AluOpType.add,
                            )
                agg = persist.tile([P, G * HID], F32, tag=agg_tag)
                for g0, g1 in GBLK:
                    nc.vector.tensor_reduce(
                        out=agg[:, g0 * HID : g1 * HID].rearrange(
                            "p (g f) -> p g f", f=HID
                        ),
                        in_=agg4[:, g0 * W * HID : g1 * W * HID].rearrange(
                            "p (g q f) -> p g f q", q=W, f=HID
                        ),
                        axis=mybir.AxisListType.X,
                        op=mybir.AluOpType.add,
                    )
                return agg

            # ---- layer 1 ----
            t1 = persist.tile([P, G * HID], F32, tag="t1")
            nc.vector.tensor_tensor(
                out=t1[:].rearrange("p (g f) -> p g f", f=HID),
                in0=H[:].rearrange("p (g f) -> p g f", f=HID),
                in1=dis_bc(HID),
                op=mybir.AluOpType.mult,
            )
            tbl4_1 = publish(t1, 0)
            agg1 = gather_reduce(tbl4_1, "agg1")

            h1 = persist.tile([P, G * HID], F32, tag="h1")
            t2 = persist.tile([P, G * HID], F32, tag="t2")
            for g0, g1 in GBLK:
                nc.vector.tensor_tensor(
                    out=sl(agg1, g0, g1, HID),
                    in0=sl(agg1, g0, g1, HID),
                    in1=sl(t1, g0, g1, HID),
                    op=mybir.AluOpType.add,
                )
                nc.vector.tensor_tensor(
                    out=sl(agg1, g0, g1, HID),
                    in0=sl(agg1, g0, g1, HID),
                    in1=dis_blk(dis, g0, g1, HID),
                    op=mybir.AluOpType.mult,
                )
                nc.vector.tensor_tensor(
                    out=sl(agg1, g0, g1, HID),
                    in0=sl(agg1, g0, g1, HID),
                    in1=b1sb[:].rearrange("p (u f) -> p u f", u=1).to_broadcast(
                        [P, g1 - g0, HID]
                    ),
                    op=mybir.AluOpType.add,
                )
                nc.scalar.activation(
                    h1[:, g0 * HID : g1 * HID],
                    agg1[:, g0 * HID : g1 * HID],
                    mybir.ActivationFunctionType.Relu,
                )
                nc.vector.tensor_tensor(
                    out=sl(t2, g0, g1, HID),
                    in0=sl(h1, g0, g1, HID),
                    in1=dis_blk(dis, g0, g1, HID),
                    op=mybir.AluOpType.mult,
                )
                nc.vector.tensor_tensor(
                    out=sl(t2, g0, g1, HID),
                    in0=sl(t2, g0, g1, HID),
                    in1=dis_blk(dmask, g0, g1, HID),
                    op=mybir.AluOpType.mult,
                )

            # ---- layer 2 ----
            tbl4_2 = publish(t2, 1)
            agg2 = gather_reduce(tbl4_2, "agg2")

            for g0, g1 in GBLK:
                nc.vector.tensor_tensor(
                    out=sl(agg2, g0, g1, HID),
                    in0=sl(agg2, g0, g1, HID),
                    in1=sl(t2, g0, g1, HID),
                    op=mybir.AluOpType.add,
                )
                nc.vector.tensor_tensor(
                    out=sl(agg2, g0, g1, HID),
                    in0=sl(agg2, g0, g1, HID),
                    in1=dis_blk(dis, g0, g1, HID),
                    op=mybir.AluOpType.mult,
                )
            V = agg2

            # ---- phase F: O = V @ W2 + b2, log_softmax ----
            phf_cm = tc.tile_pool(name="phf", bufs=1)
            phf = phf_cm.__enter__()
            O = phf.tile([P, G * OUT_CH], F32, tag="O")
            for g in range(G):
                vt_ps = psT.tile([P, P], F32, tag="pt")
                nc.tensor.transpose(
                    vt_ps[:HID, :], V[:, g * HID : (g + 1) * HID], ident[:]
                )
                vt_sb = small.tile([HID, P], F32, tag="vts")
                nc.vector.tensor_copy(vt_sb[:], vt_ps[:HID, :])
                ops = psO.tile([P, OUT_CH], F32, tag="ops")
                nc.tensor.matmul(
                    ops[:], lhsT=vt_sb[:], rhs=w2sb[:], start=True, stop=True
                )
                nc.vector.tensor_copy(O[:, g * OUT_CH : (g + 1) * OUT_CH], ops[:])

            O3 = O[:].rearrange("p (g f) -> p g f", f=OUT_CH)
            nc.vector.tensor_tensor(
                out=O3,
                in0=O3,
                in1=b2sb[:].rearrange("p (u f) -> p u f", u=1).to_broadcast(
                    [P, G, OUT_CH]
                ),
                op=mybir.AluOpType.add,
            )
            mx = small.tile([P, G], F32, tag="mx")
            nc.vector.tensor_reduce(
                out=mx[:], in_=O3, axis=mybir.AxisListType.X,
                op=mybir.AluOpType.max,
            )
            nc.vector.tensor_tensor(
                out=O3,
                in0=O3,
                in1=mx[:].rearrange("p (g u) -> p g u", u=1).to_broadcast(
                    [P, G, OUT_CH]
                ),
                op=mybir.AluOpType.subtract,
            )
            ex = phf.tile([P, G * OUT_CH], F32, tag="ex")
            nc.scalar.activation(ex[:], O[:], mybir.ActivationFunctionType.Exp)
            sm = small.tile([P, G], F32, tag="sm")
            nc.vector.tensor_reduce(
                out=sm[:],
                in_=ex[:].rearrange("p (g f) -> p g f", f=OUT_CH),
                axis=mybir.AxisListType.X,
                op=mybir.AluOpType.add,
            )
            nc.scalar.activation(sm[:], sm[:], mybir.ActivationFunctionType.Ln)
            nc.vector.tensor_tensor(
                out=O3,
                in0=O3,
                in1=sm[:].rearrange("p (g u) -> p g u", u=1).to_broadcast(
                    [P, G, OUT_CH]
                ),
                op=mybir.AluOpType.subtract,
            )
            nc.sync.dma_start(out_d[:], O[:])
            phf_cm.__exit__(None, None, None)

    nc.finalize()
    return nc


def kernel(x, W1, b1, W2, b2, edge_index, _trace=False):
    import ml_dtypes

    x = np.asarray(x, dtype=np.float32)
    W1 = np.asarray(W1, dtype=np.float32)
    b1 = np.asarray(b1, dtype=np.float32)
    W2 = np.asarray(W2, dtype=np.float32)
    b2 = np.asarray(b2, dtype=np.float32)
    edge_index = np.asarray(edge_index)

    N, IN_CH = x.shape
    HID = W1.shape[1]
    OUT_CH = W2.shape[1]

    plan = _plan(edge_index, N)
    G, NPAD, NP_ = plan["G"], plan["NPAD"], plan["NP"]
    cfg = dict(N=N, IN_CH=IN_CH, HID=HID, OUT_CH=OUT_CH)

    nc = _build(cfg, G, plan)

    m_of, p_of, g_of = plan["m_of"], plan["p_of"], plan["g_of"]
    dis_full = plan["dis"]
    w1r = np.ascontiguousarray(
        W1.reshape(IN_CH // P, P, HID).transpose(1, 0, 2).reshape(P, -1)
    )
    maps = []
    for m in range(NCORES):
        nodes = np.nonzero(m_of == np.int64(m))[0]
        lr = g_of[nodes] * P + p_of[nodes]
        xt = np.zeros((IN_CH, NPAD), dtype=ml_dtypes.bfloat16)
        xt[:, lr] = x[nodes].T.astype(ml_dtypes.bfloat16)
        disp = np.ones((P, G), dtype=np.float32)
        disp[p_of[nodes], g_of[nodes]] = dis_full[nodes].astype(np.float32)
        dmp = np.zeros((P, G), dtype=np.float32)
        dmp[p_of[nodes], g_of[nodes]] = 1.0
        maps.append(
            dict(
                xt=xt,
                w1=w1r.astype(ml_dtypes.bfloat16),
                dis=disp,
                dmask=dmp,
                b1=b1.reshape(1, HID),
                w2=W2,
                b2=b2.reshape(1, OUT_CH),
                idxt=np.tile(plan["idxt"][m], (8, 1)),
            )
        )

    exec_ns = 0
    try:
        r = run_bass_kernel_spmd(
            nc, maps, core_ids=list(range(NCORES)), trace=_trace
        )
    except ModuleNotFoundError:
        r = run_bass_kernel_spmd(
            nc, maps, core_ids=list(range(NCORES)), trace=False
        )
    if r.exec_time_ns:
        exec_ns += r.exec_time_ns
    kernel._last_result = r

    out = np.empty((N, OUT_CH), dtype=np.float32)
    for m in range(NCORES):
        Om = np.asarray(r.results[m]["out"]).reshape(P, G, OUT_CH)
        nodes = np.nonzero(m_of == np.int64(m))[0]
        out[nodes] = Om[p_of[nodes], g_of[nodes], :]

    kernel._exec_ns = exec_ns
    return out



# revision 16
# speedup vs baseline: 1.0035x; 1.0035x over previous
"""Two-layer GCN forward (GCNConv -> relu -> GCNConv -> log_softmax) on 8
Trainium2 NeuronCores - single monolithic SPMD program.

Design (pull aggregation via bulk dma_gather):
  - Nodes are in-degree-sorted globally and dealt round-robin to 8 cores;
    core-local layout is (p, g), lr = g*128 + p, global table id
    tid = m*NPAD + p*G + g.
  - Per layer the cores publish t = dis*h as a bf16 [NPAD,16] slice,
    AllGather it into the full [NTBL,16] table, and expand it locally into a
    256B-strided [NTBL,128] bf16 table (payload cols 0:16; rest junk, never
    read) because dma_gather requires 256B elements/strides and int16 ids.
  - Each core then pulls its targets' in-neighbor rows with big dma_gather
    instructions (thousands of rows each, Q7 "mlp" ucode): slots are
    organized per (source-quarter q, target group g) with uniform K(g,q)
    columns (shared schedule, group max over the 1024-node degree band);
    pad slots point at a dummy (zero) row. Gathered chunks are reduced with
    run-length uniform-K DVE tensor_reduce into per-quarter partials, then
    summed.
  - norm factorizes: table rows are dis_u*h_u; epilogue does
    dis_v*(agg + t_v) (+b). Self loops handled densely; dis = rsqrt(deg+1)
    comes from the host, as do all int16 slot tables.

kernel(**inputs) takes full unsharded inputs, returns the full [N, 40] out.
"""

import sys

import numpy as np

try:
    import concourse.bass as bass
except ImportError:  # pragma: no cover
    sys.path.insert(0, "/opt/trn_rl_repo")
    import concourse.bass as bass

import concourse.bacc as bacc
import concourse.tile as tile
from concourse import library_config, mybir
from concourse.bass_utils import run_bass_kernel_spmd
from concourse.masks import make_identity

F32 = mybir.dt.float32
BF16 = mybir.dt.bfloat16
I16 = mybir.dt.int16
P = 128
NCORES = 8
NQ = 4  # source-table quarters (int16 row-id windows)
CCH = 60  # max gather-chunk columns (tokens = 128*CCH); >=16k tokens hangs the ucode


def _plan(edge_index, N):
    row = np.asarray(edge_index[0], dtype=np.int64)
    col = np.asarray(edge_index[1], dtype=np.int64)

    NP_ = -(-N // NCORES)
    G = -(-NP_ // P)
    NPAD = G * P
    NTBL = NCORES * NPAD

    # overlapping int16 windows over the table: width WW, stride chosen so
    # most rows fall in two windows (per-edge window choice -> water-fill
    # balancing of per-target slot counts).
    WW = min(NTBL, 32752)
    if NTBL <= WW:
        W = 1
        stride = NTBL
        bases = np.array([0], dtype=np.int64)
    else:
        stride = WW // 2
        nb = -(-NTBL // stride)  # bands; band j rows -> window pair (j, j+1)
        W = nb + 1
        bases = np.clip((np.arange(W) - 1) * stride, 0, NTBL - WW)
    assert (bases + WW <= NTBL).all() and bases[-1] + WW >= NTBL

    deg_in = np.bincount(col, minlength=N).astype(np.float64)
    dis = 1.0 / np.sqrt(deg_in + 1.0)

    order = np.argsort(-deg_in, kind="stable")
    rank = np.empty(N, dtype=np.int64)
    rank[order] = np.arange(N)

    m_of = rank % NCORES
    lr_of = rank // NCORES
    g_of = lr_of // P
    p_of = lr_of % P
    tid_of = m_of * NPAD + p_of * G + g_of

    # dummy (zero) rows: one per window
    used = np.zeros(NTBL, dtype=bool)
    used[tid_of] = True
    free_rows = np.nonzero(~used)[0]
    assert free_rows.size > 0, "no dummy rows"
    dummy_of_w = np.empty(W, dtype=np.int64)
    for w in range(W):
        inw = free_rows[(free_rows >= bases[w]) & (free_rows < bases[w] + WW)]
        assert inw.size > 0, f"no dummy row in window {w}"
        dummy_of_w[w] = inw[0]

    # per-edge legal windows: band j = r//stride -> pair (j, j+1); with
    # stride = WW//2 and duplicated end windows every row has both legal.
    r = tid_of[row]
    if W == 1:
        w0 = np.zeros(len(r), dtype=np.int64)
        w1 = w0.copy()
    else:
        w0 = r // stride
        w1 = w0 + 1
        assert (r >= bases[w0]).all() and (r < bases[w0] + WW).all()
        assert (r >= bases[w1]).all() and (r < bases[w1] + WW).all()
    flex = w0 != w1  # choice between w0 and w0+1

    # water-fill per target: forced counts f[v,w], flex counts y[v,w0]
    deg = np.bincount(col, minlength=N)
    f = np.zeros((W, N), dtype=np.int64)
    y = np.zeros((W, N), dtype=np.int64)
    for w in range(W):
        f[w] = np.bincount(col[(~flex) & (w1 == w)], minlength=N)
        y[w] = np.bincount(col[flex & (w0 == w)], minlength=N)
    T = -(-deg // W)
    keep = np.zeros((W, N), dtype=np.int64)  # flex edges kept at lower window
    carry = np.zeros(N, dtype=np.int64)
    for w in range(W):
        load = f[w] + carry
        lvl = np.maximum(T, (load + y[w] + 1) // 2)  # split band spikes
        t_w = np.clip(lvl - load, 0, y[w])
        keep[w] = t_w
        carry = y[w] - t_w
    # per-edge final window: rank within (target, w0) among flex edges
    e_id = np.arange(len(row))
    fkey = np.lexsort((e_id, w0, col))
    fsel = flex[fkey]
    fk = fkey[fsel]
    grp = col[fk] * W + w0[fk]
    ch = np.empty(len(fk), dtype=bool)
    if len(fk):
        ch[0] = True
        ch[1:] = grp[1:] != grp[:-1]
        st = np.nonzero(ch)[0]
        gi = np.cumsum(ch) - 1
        rk = np.arange(len(fk)) - st[gi]
        kept = rk < keep[w0[fk], col[fk]]
        wfin = np.where(~flex, w1, 0)
        wfin[fk] = np.where(kept, w0[fk], w0[fk] + 1)
    else:
        wfin = w1.copy()

    # per (target, window) final counts -> shared K schedule
    def _counts(wf):
        cw = np.zeros((W, N), dtype=np.int64)
        for w in range(W):
            cw[w] = np.bincount(col[wf == w], minlength=N)
        K = np.zeros((W, G), dtype=np.int64)
        for w in range(W):
            cpad = np.zeros(NCORES * NPAD, dtype=np.int64)
            cpad[:N] = cw[w][order]
            K[w] = cpad.reshape(G, NCORES * P).max(axis=1)
        return cw, K

    cw, K = _counts(wfin)
    if W > 1:
        # band-max refinement: move one edge per (target, window) away from
        # ceiling windows into the adjacent legal window with headroom
        band_of = np.empty(N, dtype=np.int64)
        band_of[order] = np.arange(N) // (NCORES * P)
        alt = np.where(wfin == w0, w0 + 1, w0)
        for _ in range(4):
            Kv = K[:, band_of]
            mov = (cw[wfin, col] >= Kv[wfin, col]) & (
                Kv[alt, col] - cw[alt, col] >= 2
            )
            sel = np.nonzero(mov)[0]
            if sel.size == 0:
                break
            key = col[sel] * W + wfin[sel]
            o = np.argsort(key, kind="stable")
            sel, key = sel[o], key[o]
            first = np.ones(len(sel), dtype=bool)
            first[1:] = key[1:] != key[:-1]
            wfin[sel[first]] = alt[sel[first]]
            cw, K = _counts(wfin)

    # column layout / chunks (uniform-K runs, chunks of <= CCH columns)
    chunks = []  # (w, ncols, [(g0, g1, K, colbase)...])
    colbase_of = np.zeros((W, G), dtype=np.int64)
    for w in range(W):
        g = 0
        while g < G:
            ccols = 0
            cruns = []
            while g < G and ccols + K[w, g] <= CCH:
                k = int(K[w, g])
                g1 = g
                while g1 < G and K[w, g1] == k and ccols + (g1 - g + 1) * k <= CCH:
                    colbase_of[w, g1] = ccols + (g1 - g) * k
                    g1 += 1
                if k > 0:
                    cruns.append((g, g1, k, ccols))
                    ccols += (g1 - g) * k
                g = g1
            if not cruns and g < G:  # single group wider than CCH
                k = int(K[w, g])
                colbase_of[w, g] = 0
                cruns.append((g, g + 1, k, 0))
                ccols = k
                g += 1
            if cruns:
                chunks.append((w, ccols, cruns))
    TOTTOK = sum(c[1] for c in chunks) * P

    chunk_off = []
    off = 0
    for w, ccols, cruns in chunks:
        chunk_off.append(off)
        off += 8 * ccols
    TOTCOL = off

    idxt = np.zeros((NCORES, 16, TOTCOL), dtype=np.int16)
    for ci, (w, ccols, cruns) in enumerate(chunks):
        base = chunk_off[ci]
        j = np.arange(128 * ccols)
        idxt[:, (j % 16), base + j // 16] = dummy_of_w[w] - bases[w]
    # place real edges
    mv, pv, gv = m_of[col], p_of[col], g_of[col]
    srcl = tid_of[row] - bases[wfin]
    assert (srcl >= 0).all() and (srcl < WW).all()
    ekey = np.lexsort((row, gv, pv, wfin, mv))
    smv, spv, sgv, swv = mv[ekey], pv[ekey], gv[ekey], wfin[ekey]
    ssrc = srcl[ekey]
    grp = ((smv * P + spv) * G + sgv) * W + swv
    changes = np.empty(len(grp), dtype=bool)
    changes[0] = True
    changes[1:] = grp[1:] != grp[:-1]
    starts = np.nonzero(changes)[0]
    gid = np.cumsum(changes) - 1
    k_of = np.arange(len(grp)) - starts[gid]
    chunk_of_wg = np.zeros((W, G), dtype=np.int64)
    for ci, (w, ccols, cruns) in enumerate(chunks):
        for (g0, g1, k, cb) in cruns:
            chunk_of_wg[w, g0:g1] = ci
    cidx = chunk_of_wg[swv, sgv]
    colc = colbase_of[swv, sgv] + k_of
    j = colc * 128 + spv
    coff = np.asarray(chunk_off)[cidx]
    idxt[smv, j % 16, coff + j // 16] = ssrc.astype(np.int16)

    return dict(
        NP=NP_, G=G, NPAD=NPAD, NTBL=NTBL, W=W, WW=WW, bases=bases,
        TOTCOL=TOTCOL, TOTTOK=TOTTOK, chunks=chunks, chunk_off=chunk_off,
        order=order, dis=dis, m_of=m_of, p_of=p_of, g_of=g_of, idxt=idxt,
    )


def _build(cfg, G, plan):
    IN_CH, HID, OUT_CH = cfg["IN_CH"], cfg["HID"], cfg["OUT_CH"]
    NPAD = G * P
    NTBL, W, WW = plan["NTBL"], plan["W"], plan["WW"]
    bases = plan["bases"]
    TOTCOL = plan["TOTCOL"]
    chunks, chunk_off = plan["chunks"], plan["chunk_off"]
    MAXC = max(c[1] for c in chunks)
    NC_IN = IN_CH // P
    XCHUNK = next(d for d in range(1, G + 1) if G % d == 0 and G // d <= 16)
    GH = G // XCHUNK

    nc = bacc.Bacc(None, num_devices=NCORES, num_swdge_queues=4)

    xt_d = nc.dram_tensor("xt", [IN_CH, NPAD], BF16, kind="ExternalInput")
    w1_d = nc.dram_tensor("w1", [P, NC_IN * HID], BF16, kind="ExternalInput")
    dis_d = nc.dram_tensor("dis", [P, G], F32, kind="ExternalInput")
    dmask_d = nc.dram_tensor("dmask", [P, G], F32, kind="ExternalInput")
    b1_d = nc.dram_tensor("b1", [1, HID], F32, kind="ExternalInput")
    w2_d = nc.dram_tensor("w2", [HID, OUT_CH], F32, kind="ExternalInput")
    b2_d = nc.dram_tensor("b2", [1, OUT_CH], F32, kind="ExternalInput")
    idx_d = nc.dram_tensor("idxt", [P, TOTCOL], I16, kind="ExternalInput")
    out_d = nc.dram_tensor("out", [P, G * OUT_CH], F32, kind="ExternalOutput")

    groups = [list(range(NCORES))]

    with tile.TileContext(nc) as tc:
        nc.gpsimd.load_library(library_config.mlp)
        with (
            tc.tile_pool(name="const", bufs=1) as const,
            tc.tile_pool(name="persist", bufs=1) as persist,
            tc.tile_pool(name="small", bufs=2) as small,
            tc.tile_pool(name="dram", bufs=1, space="DRAM") as dram,
            tc.tile_pool(name="psH", bufs=4, space="PSUM") as psH,
            tc.tile_pool(name="psT", bufs=2, space="PSUM") as psT,
            tc.tile_pool(name="psO", bufs=2, space="PSUM") as psO,
        ):
            # ---- constants ----
            ident = const.tile([P, P], F32)
            make_identity(nc, ident[:])
            warm = psT.tile([P, P], F32, tag="pt")
            nc.tensor.transpose(warm[:], ident[:], ident[:])
            w1sb = const.tile([P, NC_IN, HID], BF16)
            nc.sync.dma_start(
                w1sb[:], w1_d[:].rearrange("k (c f) -> k c f", f=HID)
            )
            w2st = small.tile([HID, OUT_CH], F32, tag="vts")
            nc.sync.dma_start(w2st[:], w2_d[:])
            w2sb = const.tile([HID, OUT_CH], F32)
            nc.vector.tensor_copy(w2sb[:], w2st[:])
            b1sb = const.tile([P, HID], F32)
            nc.sync.dma_start(b1sb[:], b1_d[:].to_broadcast([P, HID]))
            b2sb = const.tile([P, OUT_CH], F32)
            nc.sync.dma_start(b2sb[:], b2_d[:].to_broadcast([P, OUT_CH]))
            dis = const.tile([P, G], F32)
            nc.sync.dma_start(dis[:], dis_d[:])
            dmask = const.tile([P, G], F32)
            nc.sync.dma_start(dmask[:], dmask_d[:])

            def dis_bc(F):
                return (
                    dis[:]
                    .rearrange("p (g u) -> p g u", u=1)
                    .to_broadcast([P, G, F])
                )

            # ---- phase A: H = X @ W1 via pretransposed bf16 panels ----
            H = persist.tile([P, G * HID], F32, tag="H")
            with tc.tile_pool(name="xp", bufs=2) as xpp:
                for ch in range(XCHUNK):
                    xp = xpp.tile([P, NC_IN, GH * P], BF16, tag="xp")
                    nc.sync.dma_start(
                        xp[:],
                        xt_d[:, ch * GH * P : (ch + 1) * GH * P].rearrange(
                            "(c k) n -> k c n", k=P
                        ),
                    )
                    for gl in range(GH):
                        g = ch * GH + gl
                        hps = psH.tile([P, HID], F32, tag="hps")
                        for c in range(NC_IN):
                            nc.tensor.matmul(
                                hps[:],
                                lhsT=xp[:, c, gl * P : (gl + 1) * P],
                                rhs=w1sb[:, c, :],
                                start=(c == 0),
                                stop=(c == NC_IN - 1),
                            )
                        nc.vector.tensor_copy(
                            H[:, g * HID : (g + 1) * HID], hps[:]
                        )

            def publish(tsrc, layer):
                """t [P,G*HID] f32 -> bf16 slice -> AllGather -> 256B table."""
                tbf = small.tile([P, G * HID], BF16, tag="tbf")
                nc.vector.tensor_copy(tbf[:], tsrc[:])
                bounce = dram.tile([NPAD, HID], BF16, tag=f"bounce{layer}")
                nc.sync.dma_start(
                    bounce[:].rearrange("(p g) f -> p (g f)", p=P), tbf[:]
                )
                tbl = dram.tile(
                    [NTBL, HID], BF16, tag=f"tbl{layer}", addr_space="Shared"
                )
                nc.gpsimd.collective_compute(
                    "AllGather",
                    mybir.AluOpType.bypass,
                    ins=[bounce[:].opt()],
                    outs=[tbl[:].opt()],
                    replica_groups=groups,
                )
                tbl4 = dram.tile([NTBL, P], BF16, tag=f"tbl4{layer}")
                NEX = -(-NTBL // 65536) + 1
                step = -(-NTBL // NEX)
                for e in range(NEX):
                    lo, hi = e * step, min((e + 1) * step, NTBL)
                    nc.sync.dma_start(
                        tbl4[lo:hi, :HID], tbl[lo:hi, :]
                    )
                return tbl4

            def gather_reduce(tbl4, agg_tag, gdp, ixp, layer):
                agg4 = persist.tile([P, G * W * HID], F32, tag="agg4")
                nc.vector.memset(agg4[:], 0.0)
                a4 = agg4[:].rearrange("p (g q f) -> p g q f", q=W, f=HID)
                if True:
                    for ci, (q, ccols, cruns) in enumerate(chunks):
                        cj = ci + layer * len(chunks)
                        T = P * ccols
                        ist = ixp.tile([P, 8 * MAXC], I16, tag=f"ist{cj % 4}")
                        nc.sync.dma_start(
                            ist[:, : 8 * ccols],
                            idx_d[:, chunk_off[ci] : chunk_off[ci] + 8 * ccols],
                        )
                        gd = gdp.tile([P, MAXC, P], BF16, tag=f"gd{cj % 6}")
                        nc.gpsimd.dma_gather(
                            gd[:, :ccols, :],
                            tbl4[int(bases[q]) : int(bases[q]) + WW, :],
                            ist[:, : 8 * ccols],
                            T,
                            T,
                            P,
                            single_packet=False,
                            queue_num=cj % 4,
                        )
                        for (g0, g1, k, cb) in cruns:
                            nc.vector.tensor_reduce(
                                out=a4[:, g0:g1, q, :],
                                in_=gd[:, cb : cb + (g1 - g0) * k, :HID].rearrange(
                                    "p (g k) f -> p g f k", k=k
                                ),
                                axis=mybir.AxisListType.X,
                                op=mybir.AluOpType.add,
                            )
                agg = persist.tile([P, G * HID], F32, tag=agg_tag)
                nc.vector.tensor_reduce(
                    out=agg[:].rearrange("p (g f) -> p g f", f=HID),
                    in_=agg4[:].rearrange("p (g q f) -> p g f q", q=W, f=HID),
                    axis=mybir.AxisListType.X,
                    op=mybir.AluOpType.add,
                )
                return agg

            # ---- layer 1 ----
            gdp_cm = tc.tile_pool(name="gd", bufs=1)
            gdp = gdp_cm.__enter__()
            ixp_cm = tc.tile_pool(name="ix", bufs=2)
            ixp = ixp_cm.__enter__()
            t1 = persist.tile([P, G * HID], F32, tag="t1")
            nc.vector.tensor_tensor(
                out=t1[:].rearrange("p (g f) -> p g f", f=HID),
                in0=H[:].rearrange("p (g f) -> p g f", f=HID),
                in1=dis_bc(HID),
                op=mybir.AluOpType.mult,
            )
            tbl4_1 = publish(t1, 0)
            agg1 = gather_reduce(tbl4_1, "agg1", gdp, ixp, 0)

            nc.vector.tensor_tensor(
                out=agg1[:], in0=agg1[:], in1=t1[:], op=mybir.AluOpType.add
            )
            nc.vector.tensor_tensor(
                out=agg1[:].rearrange("p (g f) -> p g f", f=HID),
                in0=agg1[:].rearrange("p (g f) -> p g f", f=HID),
                in1=dis_bc(HID),
                op=mybir.AluOpType.mult,
            )
            nc.vector.tensor_tensor(
                out=agg1[:].rearrange("p (g f) -> p g f", f=HID),
                in0=agg1[:].rearrange("p (g f) -> p g f", f=HID),
                in1=b1sb[:].rearrange("p (u f) -> p u f", u=1).to_broadcast(
                    [P, G, HID]
                ),
                op=mybir.AluOpType.add,
            )
            h1 = persist.tile([P, G * HID], F32, tag="h1")
            nc.scalar.activation(
                h1[:], agg1[:], mybir.ActivationFunctionType.Relu
            )

            # ---- layer 2 ----
            t2 = persist.tile([P, G * HID], F32, tag="t2")
            nc.vector.tensor_tensor(
                out=t2[:].rearrange("p (g f) -> p g f", f=HID),
                in0=h1[:].rearrange("p (g f) -> p g f", f=HID),
                in1=dis_bc(HID),
                op=mybir.AluOpType.mult,
            )
            nc.vector.tensor_tensor(
                out=t2[:].rearrange("p (g f) -> p g f", f=HID),
                in0=t2[:].rearrange("p (g f) -> p g f", f=HID),
                in1=dmask[:].rearrange("p (g u) -> p g u", u=1).to_broadcast(
                    [P, G, HID]
                ),
                op=mybir.AluOpType.mult,
            )
            tbl4_2 = publish(t2, 1)
            agg2 = gather_reduce(tbl4_2, "agg2", gdp, ixp, 1)

            nc.vector.tensor_tensor(
                out=agg2[:], in0=agg2[:], in1=t2[:], op=mybir.AluOpType.add
            )
            nc.vector.tensor_tensor(
                out=agg2[:].rearrange("p (g f) -> p g f", f=HID),
                in0=agg2[:].rearrange("p (g f) -> p g f", f=HID),
                in1=dis_bc(HID),
                op=mybir.AluOpType.mult,
            )
            V = agg2

            # ---- phase F: O = V @ W2 + b2, log_softmax ----
            ixp_cm.__exit__(None, None, None)
            gdp_cm.__exit__(None, None, None)
            phf_cm = tc.tile_pool(name="phf", bufs=1)
            phf = phf_cm.__enter__()
            O = phf.tile([P, G * OUT_CH], F32, tag="O")
            for g in range(G):
                vt_ps = psT.tile([P, P], F32, tag="pt")
                nc.tensor.transpose(
                    vt_ps[:HID, :], V[:, g * HID : (g + 1) * HID], ident[:]
                )
                vt_sb = small.tile([HID, P], F32, tag="vts")
                nc.vector.tensor_copy(vt_sb[:], vt_ps[:HID, :])
                ops = psO.tile([P, OUT_CH], F32, tag="ops")
                nc.tensor.matmul(
                    ops[:], lhsT=vt_sb[:], rhs=w2sb[:], start=True, stop=True
                )
                nc.vector.tensor_copy(O[:, g * OUT_CH : (g + 1) * OUT_CH], ops[:])

            O3 = O[:].rearrange("p (g f) -> p g f", f=OUT_CH)
            nc.vector.tensor_tensor(
                out=O3,
                in0=O3,
                in1=b2sb[:].rearrange("p (u f) -> p u f", u=1).to_broadcast(
                    [P, G, OUT_CH]
                ),
                op=mybir.AluOpType.add,
            )
            mx = small.tile([P, G], F32, tag="mx")
            nc.vector.tensor_reduce(
                out=mx[:], in_=O3, axis=mybir.AxisListType.X,
                op=mybir.AluOpType.max,
            )
            nc.vector.tensor_tensor(
                out=O3,
                in0=O3,
                in1=mx[:].rearrange("p (g u) -> p g u", u=1).to_broadcast(
                    [P, G, OUT_CH]
                ),
                op=mybir.AluOpType.subtract,
            )
            ex = phf.tile([P, G * OUT_CH], F32, tag="ex")
            nc.scalar.activation(ex[:], O[:], mybir.ActivationFunctionType.Exp)
            sm = small.tile([P, G], F32, tag="sm")
            nc.vector.tensor_reduce(
                out=sm[:],
                in_=ex[:].rearrange("p (g f) -> p g f", f=OUT_CH),
                axis=mybir.AxisListType.X,
                op=mybir.AluOpType.add,
            )
            nc.scalar.activation(sm[:], sm[:], mybir.ActivationFunctionType.Ln)
            nc.vector.tensor_tensor(
                out=O3,
                in0=O3,
                in1=sm[:].rearrange("p (g u) -> p g u", u=1).to_broadcast(
                    [P, G, OUT_CH]
                ),
                op=mybir.AluOpType.subtract,
            )
            nc.sync.dma_start(out_d[:], O[:])
            phf_cm.__exit__(None, None, None)

    nc.finalize()
    return nc


def kernel(x, W1, b1, W2, b2, edge_index, _trace=False):
    import ml_dtypes

    x = np.asarray(x, dtype=np.float32)
    W1 = np.asarray(W1, dtype=np.float32)
    b1 = np.asarray(b1, dtype=np.float32)
    W2 = np.asarray(W2, dtype=np.float32)
    b2 = np.asarray(b2, dtype=np.float32)
    edge_index = np.asarray(edge_index)

    N, IN_CH = x.shape
    HID = W1.shape[1]
    OUT_CH = W2.shape[1]

    plan = _plan(edge_index, N)
    G, NPAD, NP_ = plan["G"], plan["NPAD"], plan["NP"]
    cfg = dict(N=N, IN_CH=IN_CH, HID=HID, OUT_CH=OUT_CH)

    nc = _build(cfg, G, plan)

    m_of, p_of, g_of = plan["m_of"], plan["p_of"], plan["g_of"]
    dis_full = plan["dis"]
    w1r = np.ascontiguousarray(
        W1.reshape(IN_CH // P, P, HID).transpose(1, 0, 2).reshape(P, -1)
    )
    maps = []
    for m in range(NCORES):
        nodes = np.nonzero(m_of == np.int64(m))[0]
        lr = g_of[nodes] * P + p_of[nodes]
        xt = np.zeros((IN_CH, NPAD), dtype=ml_dtypes.bfloat16)
        xt[:, lr] = x[nodes].T.astype(ml_dtypes.bfloat16)
        disp = np.ones((P, G), dtype=np.float32)
        disp[p_of[nodes], g_of[nodes]] = dis_full[nodes].astype(np.float32)
        dmp = np.zeros((P, G), dtype=np.float32)
        dmp[p_of[nodes], g_of[nodes]] = 1.0
        maps.append(
            dict(
                xt=xt,
                w1=w1r.astype(ml_dtypes.bfloat16),
                dis=disp,
                dmask=dmp,
                b1=b1.reshape(1, HID),
                w2=W2,
                b2=b2.reshape(1, OUT_CH),
                idxt=np.tile(plan["idxt"][m], (8, 1)),
            )
        )

    exec_ns = 0
    try:
        r = run_bass_kernel_spmd(
            nc, maps, core_ids=list(range(NCORES)), trace=_trace
        )
    except ModuleNotFoundError:
        r = run_bass_kernel_spmd(
            nc, maps, core_ids=list(range(NCORES)), trace=False
        )
    if r.exec_time_ns:
        exec_ns += r.exec_time_ns
    kernel._last_result = r

    out = np.empty((N, OUT_CH), dtype=np.float32)
    for m in range(NCORES):
        Om = np.asarray(r.results[m]["out"]).reshape(P, G, OUT_CH)
        nodes = np.nonzero(m_of == np.int64(m))[0]
        out[nodes] = Om[p_of[nodes], g_of[nodes], :]

    kernel._exec_ns = exec_ns
    return out

